# revision 24
# baseline (speedup 1.0000x reference)
"""Local window attention (7x7 windows, 8 heads, d=64) Trainium2 Bass kernel, v3.

Full inputs in, full outputs out. Data-parallel over batch: 4 images/core x 8.
  fmap (32,56,56,256) f32, Wq (256,512), Wkv (256,1024), Wo (512,256), bo (256,)

v3 dataflow (channel-major, host pre-transposed):
  Host packs fmap to fmp[img, ck, c128, wx, wy, t64] bf16 (t = p1*7+p2, 64-slot
  padded windows, zeros in pad) so the kernel DMAs f^T directly - no on-chip
  input transposes.
  Per (img, wx) tile (512 padded tokens = 8 windows = 4 window-pairs):
    qT/kT [n128 x4, t512] = Wq/Wk chunk.T @ fT      (16 MMs, N=512)
    v     [t128, n512] per pair = fT-slice.T @ Wv   (8 MMs, N=512)
    per pair: S^T pair-batched [j128, hp, nk, i128] (8 MMs, N=128,
        row-tiled (64*hp,0); off-diagonal window blocks are garbage, unused)
      exp of the two diagonal blocks (2 ACTs) -> expS [j128, hp, nk, i64] bf16
      AV: av[i64w.., h, 0:65] = expS_w.T @ [v|1]    (16 MMs, N=65,
        quadrant-tiled (64w,64w); col 64 = softmax denominator)
      out_tok = av * recip(denom) broadcast        (1 recip + 1 tensor_tensor)
      outT = PE-transpose(out_tok)                 (4 transposes)
      fin [t128, 256] = outT.T @ Wo + bo           (5 MMs, N=256) -> DMA out
  Host unpacks [img, wx, u, t128, c] bf16 -> (32,56,56,256) f32.
"""

from contextlib import ExitStack

import ml_dtypes
import numpy as np

import concourse.bacc as bacc
import concourse.bass as bass
import concourse.tile as tile
from concourse import mybir
from concourse.masks import make_identity
from concourse.bass_utils import run_bass_kernel_spmd

P = 7
PP = 49
H = 8
D = 64
DIM = 256
INNER = 512
SCALE = D ** -0.5
IMGS_PER_CORE = 4
NCORES = 8
X = 56
NW = X // P      # 8 windows per axis
FP32 = mybir.dt.float32
BF16 = mybir.dt.bfloat16
NPBF16 = ml_dtypes.bfloat16
FP8 = mybir.dt.float8e4
NPFP8 = ml_dtypes.float8_e4m3
WSCALE = 8.0   # fp8 weight prescale; folded back out via exp scale / fo scale
Exp = mybir.ActivationFunctionType.Exp
DR = mybir.MatmulPerfMode.DoubleRow


def build_bass(n_imgs=IMGS_PER_CORE):
    nc = bacc.Bacc("TRN2", target_bir_lowering=False, debug=False)

    fmp = nc.dram_tensor("fmp", [n_imgs, 2, 128, NW, NW, 64], BF16,
                         kind="ExternalInput").ap()
    wq = nc.dram_tensor("Wq", [2, 128, INNER], BF16, kind="ExternalInput").ap()
    wk = nc.dram_tensor("Wk", [2, 128, INNER], BF16, kind="ExternalInput").ap()
    wv = nc.dram_tensor("Wv", [2, 128, INNER], BF16, kind="ExternalInput").ap()
    wo = nc.dram_tensor("Wo", [4, 128, DIM], BF16, kind="ExternalInput").ap()
    bo = nc.dram_tensor("bo", [DIM], FP32, kind="ExternalInput").ap()
    out = nc.dram_tensor("out", [n_imgs, NW, 128, 2, 512], BF16,
                         kind="ExternalOutput").ap()

    with tile.TileContext(nc) as tc:
        with ExitStack() as ctx:
            build_kernel(ctx, tc, out, fmp, wq, wk, wv, wo, bo, n_imgs)
    nc.compile()
    return nc


def build_kernel(ctx, tc, out, fmp, wq, wk, wv, wo, bo, n_imgs):
    nc = tc.nc
    consts = ctx.enter_context(tc.tile_pool(name="consts", bufs=1))
    sb = ctx.enter_context(tc.tile_pool(name="sb", bufs=3))
    ps = ctx.enter_context(tc.tile_pool(name="ps", bufs=4, space="PSUM"))

    ident = consts.tile([128, 128], BF16)
    make_identity(nc, ident[:])

    wq_s = consts.tile([128, 2, INNER], BF16)
    nc.sync.dma_start(out=wq_s[:], in_=wq.rearrange("kc ck n -> ck kc n"))
    wk_s = consts.tile([128, 2, INNER], BF16)
    nc.sync.dma_start(out=wk_s[:], in_=wk.rearrange("kc ck n -> ck kc n"))
    wv_s = consts.tile([128, 2, INNER], BF16)
    nc.sync.dma_start(out=wv_s[:], in_=wv.rearrange("kc ck n -> ck kc n"))
    wo_s = consts.tile([128, 4, DIM], BF16)
    nc.sync.dma_start(out=wo_s[:], in_=wo.rearrange("kc ck m -> ck kc m"))
    bo_s = consts.tile([128, 2], FP32)
    nc.sync.dma_start(out=bo_s[:], in_=bo.rearrange("(cc p) -> p cc", p=128))

    def ps_slot(shape, dtype=FP32):
        # uniform 2-bank (4KB/partition) psum slots; view-slice to shape
        t = ps.tile([128, 4096 // mybir.dt.size(dtype)], dtype, tag="ps")
        n = int(np.prod(shape[1:]))
        v_ = t[:, 0:n]
        if len(shape) > 2:
            dims = " ".join(f"d{i}" for i in range(1, len(shape)))
            v_ = v_.rearrange(f"p ({dims}) -> p {dims}",
                              **{f"d{i}": shape[i] for i in range(1, len(shape) - 1)})
        return v_

    prev = None
    for img in range(n_imgs):
        for wx in range(NW):
            vhat, qT, kT = tile_qkv(nc, sb, ps_slot, fmp, wq_s, wk_s, wv_s,
                                    img, wx)
            outT_tile = sb.tile([128, 4, 512], BF16, tag="outT_tile")
            ts = (outT_tile, img, wx)
            for u in range(4):
                expS = attn_phase_s(nc, sb, ps_slot, qT, kT, u)
                if prev is not None:
                    pair_out_phase(nc, sb, ps_slot, ident, prev)
                out_tok = attn_phase_av(nc, sb, ps_slot, expS, vhat[u])
                if prev is not None and prev[2] == 3:
                    tile_out_phase(nc, sb, ps_slot, out, wo_s, bo_s, prev[1])
                prev = (out_tok, ts, u)
    pair_out_phase(nc, sb, ps_slot, ident, prev)
    tile_out_phase(nc, sb, ps_slot, out, wo_s, bo_s, prev[1])


def tile_qkv(nc, sb, ps_slot, fmp, wq_s, wk_s, wv_s, img, wx):
    # ---- load fT [c128, ck, t512] ----
    fT = sb.tile([128, 2, 512], BF16, tag="fT")
    nc.sync.dma_start(
        out=fT[:],
        in_=fmp[img, :, :, wx, :, :].rearrange("ck c wy t -> c ck (wy t)"))

    # ---- qT, kT: [n128, t512] x4 chunks ----
    qp = [ps_slot([128, 2, 512]) for _ in range(2)]   # slot holds nk, nk+1
    kp = [ps_slot([128, 2, 512]) for _ in range(2)]
    for half in range(2):
        for sub in range(2):
            nk = 2 * half + sub
            for kc in range(2):
                nc.tensor.matmul(qp[half][:, sub, :],
                                 wq_s[:, kc, 128 * nk:128 * nk + 128],
                                 fT[:, kc, :], start=(kc == 0), stop=(kc == 1))
        for sub in range(2):
            nk = 2 * half + sub
            for kc in range(2):
                nc.tensor.matmul(kp[half][:, sub, :],
                                 wk_s[:, kc, 128 * nk:128 * nk + 128],
                                 fT[:, kc, :], start=(kc == 0), stop=(kc == 1))
    qT = sb.tile([128, 4, 512], BF16, tag="qT")
    kT = sb.tile([128, 4, 512], BF16, tag="kT")
    # q/k evacuation on scalar; vector carries vhat/norm/outT
    nc.scalar.copy(qT[:, 0:2, :], qp[0][:])
    nc.scalar.copy(qT[:, 2:4, :], qp[1][:])
    nc.scalar.copy(kT[:, 0:2, :], kp[0][:])
    nc.scalar.copy(kT[:, 2:4, :], kp[1][:])

    # ---- v: per pair [t128, n512]; vhat adds ones col per head ----
    vhat = []
    for uh in range(2):
        vp = ps_slot([128, 2, 512])   # two pairs per slot? no: [t128, pairsub, n512]
        for us in range(2):
            u = 2 * uh + us
            for kc in range(2):
                nc.tensor.matmul(vp[:, us, :],
                                 fT[:, kc, 128 * u:128 * u + 128],
                                 wv_s[:, kc, :], start=(kc == 0), stop=(kc == 1))
        for us in range(2):
            u = 2 * uh + us
            vh = sb.tile([128, H, D + 1], BF16, tag=f"vhat{u}")
            nc.vector.tensor_copy(
                vh[:, :, 0:D], vp[:, us, :].rearrange("p (h d) -> p h d", h=H))
            nc.gpsimd.memset(vh[:, :, D:D + 1], 1.0)
            vhat.append(vh)

    return vhat, qT, kT


def attn_phase_s(nc, sb, ps_slot, qT, kT, u):
    """Quad-tiled S^T + one full-width exp."""
    # sp[j128, hp, nkx8, i64]: hp stride = 8*64*4B = 2KB -> bank-separated
    sp = ps_slot([128, 2, 8, 64])
    for nk in range(4):
        for hp in range(2):
            for w in range(2):
                o = 128 * u + 64 * w
                nc.tensor.matmul(
                    sp[64 * w:64 * w + 64, hp, nk, :],
                    kT[64 * hp:64 * hp + 64, nk, o:o + 64],
                    qT[64 * hp:64 * hp + 64, nk, o:o + 64],
                    start=True, stop=True, tile_position=(64 * hp, 64 * w))
    expS = sb.tile([128, 2, 4, 64], BF16, tag="expS")
    nc.scalar.activation(expS[:], sp[:, :, 0:4, :], Exp, scale=SCALE)
    return expS


def attn_phase_av(nc, sb, ps_slot, expS, vh):
    """AV + denominator, normalize. Returns out_tok [i128, h, d] bf16."""
    av = ps_slot([128, H, 128])
    for h in range(H):
        nk, hp = h // 2, h % 2
        for w in range(2):
            nc.tensor.matmul(
                av[64 * w:64 * w + 64, h, 0:D + 1],
                expS[64 * w:64 * w + PP, hp, nk, :],
                vh[64 * w:64 * w + PP, h, :],
                start=True, stop=True, tile_position=(64 * w, 64 * w))
    recd = sb.tile([128, H], FP32, tag="recd")
    nc.vector.reciprocal(recd[:], av[:, :, D])
    out_tok = sb.tile([128, H, D], BF16, tag="out_tok")
    nc.vector.tensor_tensor(
        out=out_tok[:], in0=av[:, :, 0:D],
        in1=recd[:].unsqueeze(2).broadcast_to([128, H, D]),
        op=mybir.AluOpType.mult)
    return out_tok


def pair_out_phase(nc, sb, ps_slot, ident, prev):
    """Transpose a previous pair's out_tok into its tile's outT buffer."""
    out_tok, (outT_tile, img, wx), u = prev
    tp = ps_slot([128, 4, 128], BF16)
    ot2 = out_tok[:].rearrange("p h d -> p (h d)")
    for nk in range(4):
        nc.tensor.transpose(tp[:, nk, :], ot2[:, 128 * nk:128 * nk + 128],
                            ident[:])
    nc.vector.tensor_copy(outT_tile[:, :, 128 * u:128 * u + 128], tp[:])


def tile_out_phase(nc, sb, ps_slot, out, wo_s, bo_s, ts):
    """Channel-major out-projection for a whole tile + bias + store."""
    outT_tile, img, wx = ts
    fp_ = ps_slot([128, 2, 512])
    for cc in range(2):
        for nk in range(4):
            nc.tensor.matmul(fp_[:, cc, :],
                             wo_s[:, nk, 128 * cc:128 * cc + 128],
                             outT_tile[:, nk, :],
                             start=(nk == 0), stop=(nk == 3))
    fo = sb.tile([128, 2, 512], BF16, tag="fo")
    for cc in range(2):
        nc.vector.tensor_scalar(
            out=fo[:, cc, :], in0=fp_[:, cc, :],
            scalar1=bo_s[:, cc:cc + 1], scalar2=None,
            op0=mybir.AluOpType.add)
    nc.sync.dma_start(out=out[img, wx], in_=fo[:])


_CACHED = {}


def _get_nc():
    if "nc" not in _CACHED:
        _CACHED["nc"] = build_bass()
    return _CACHED["nc"]


def _marshal_fmap(fmap):
    b = fmap.shape[0]
    A = fmap.reshape(b, NW, P, NW, P, DIM).transpose(0, 5, 1, 3, 2, 4)
    A = np.ascontiguousarray(A).reshape(b, DIM, NW, NW, PP)
    T = np.zeros((b, DIM, NW, NW, 64), dtype=np.float32)
    T[..., :PP] = A
    T = T.reshape(b, 2, 128, NW, NW, 64)
    return T.astype(NPBF16)


def _unmarshal_out(O, b):
    # O: [b, wx, cp128, cc2, t512] bf16, channel-major
    V = O.transpose(0, 1, 3, 2, 4).reshape(b, NW, DIM, 4, 2, 64)[..., :PP]
    V = V.reshape(b, NW, DIM, NW, P, P)          # img, wx, c, wy, p1, p2
    V = V.transpose(0, 1, 4, 3, 5, 2).reshape(b, X, X, DIM)
    return V.astype(np.float32)


def kernel(fmap, Wq, Wkv, Wo, bo, _trace=False, _trace_kwargs=None):
    fmp = _marshal_fmap(np.ascontiguousarray(fmap))
    Wq_ = np.ascontiguousarray(Wq).astype(NPBF16).reshape(2, 128, INNER)
    Wk_ = np.ascontiguousarray(Wkv[:, :INNER]).astype(NPBF16).reshape(2, 128, INNER)
    Wv_ = np.ascontiguousarray(Wkv[:, INNER:]).astype(NPBF16).reshape(2, 128, INNER)
    Wo_ = np.ascontiguousarray(Wo).astype(NPBF16).reshape(4, 128, DIM)
    bo_ = np.ascontiguousarray(bo).astype(np.float32)
    nc = _get_nc()
    in_maps = []
    for c in range(NCORES):
        in_maps.append({
            "fmp": fmp[IMGS_PER_CORE * c:IMGS_PER_CORE * (c + 1)],
            "Wq": Wq_, "Wk": Wk_, "Wv": Wv_, "Wo": Wo_, "bo": bo_,
        })
    res = run_bass_kernel_spmd(nc, in_maps, core_ids=list(range(NCORES)),
                               trace=_trace, **(_trace_kwargs or {}))
    outs = [_unmarshal_out(r["out"], IMGS_PER_CORE) for r in res.results]
    full = np.concatenate(outs, axis=0)
    if _trace:
        return full, res
    return full


# revision 25
# speedup vs baseline: 1.1187x; 1.1187x over previous
"""Local window attention (7x7 windows, 8 heads, d=64) Trainium2 Bass kernel, v3.

Full inputs in, full outputs out. Data-parallel over batch: 4 images/core x 8.
  fmap (32,56,56,256) f32, Wq (256,512), Wkv (256,1024), Wo (512,256), bo (256,)

v3 dataflow (channel-major, host pre-transposed):
  Host packs fmap to fmp[img, ck, c128, wx, wy, t64] bf16 (t = p1*7+p2, 64-slot
  padded windows, zeros in pad) so the kernel DMAs f^T directly - no on-chip
  input transposes.
  Per (img, wx) tile (512 padded tokens = 8 windows = 4 window-pairs):
    qT/kT [n128 x4, t512] = Wq/Wk chunk.T @ fT      (16 MMs, N=512)
    v     [t128, n512] per pair = fT-slice.T @ Wv   (8 MMs, N=512)
    per pair: S^T pair-batched [j128, hp, nk, i128] (8 MMs, N=128,
        row-tiled (64*hp,0); off-diagonal window blocks are garbage, unused)
      exp of the two diagonal blocks (2 ACTs) -> expS [j128, hp, nk, i64] bf16
      AV: av[i64w.., h, 0:65] = expS_w.T @ [v|1]    (16 MMs, N=65,
        quadrant-tiled (64w,64w); col 64 = softmax denominator)
      out_tok = av * recip(denom) broadcast        (1 recip + 1 tensor_tensor)
      outT = PE-transpose(out_tok)                 (4 transposes)
      fin [t128, 256] = outT.T @ Wo + bo           (5 MMs, N=256) -> DMA out
  Host unpacks [img, wx, u, t128, c] bf16 -> (32,56,56,256) f32.
"""

from contextlib import ExitStack

import ml_dtypes
import numpy as np

import concourse.bacc as bacc
import concourse.bass as bass
import concourse.tile as tile
from concourse import mybir
from concourse.masks import make_identity
from concourse.bass_utils import run_bass_kernel_spmd

P = 7
PP = 49
H = 8
D = 64
DIM = 256
INNER = 512
SCALE = D ** -0.5
IMGS_PER_CORE = 4
NCORES = 8
X = 56
NW = X // P      # 8 windows per axis
FP32 = mybir.dt.float32
BF16 = mybir.dt.bfloat16
NPBF16 = ml_dtypes.bfloat16
FP8 = mybir.dt.float8e4
NPFP8 = ml_dtypes.float8_e4m3
WSCALE = 8.0   # fp8 weight prescale; folded back out via exp scale / fo scale
Exp = mybir.ActivationFunctionType.Exp
DR = mybir.MatmulPerfMode.DoubleRow


def build_bass(n_imgs=IMGS_PER_CORE):
    nc = bacc.Bacc("TRN2", target_bir_lowering=False, debug=False)

    fmp = nc.dram_tensor("fmp", [n_imgs, 2, 128, NW, NW, 64], BF16,
                         kind="ExternalInput").ap()
    wq = nc.dram_tensor("Wq", [2, 128, INNER], BF16, kind="ExternalInput").ap()
    wk = nc.dram_tensor("Wk", [2, 128, INNER], BF16, kind="ExternalInput").ap()
    wv = nc.dram_tensor("Wv", [2, 128, INNER], BF16, kind="ExternalInput").ap()
    wo = nc.dram_tensor("Wo", [4, 128, DIM], BF16, kind="ExternalInput").ap()
    bo = nc.dram_tensor("bo", [DIM], FP32, kind="ExternalInput").ap()
    out = nc.dram_tensor("out", [n_imgs, NW, 128, 2, 512], BF16,
                         kind="ExternalOutput").ap()

    with tile.TileContext(nc) as tc:
        with ExitStack() as ctx:
            build_kernel(ctx, tc, out, fmp, wq, wk, wv, wo, bo, n_imgs)
    nc.compile()
    return nc


def build_kernel(ctx, tc, out, fmp, wq, wk, wv, wo, bo, n_imgs):
    nc = tc.nc
    consts = ctx.enter_context(tc.tile_pool(name="consts", bufs=1))
    sb = ctx.enter_context(tc.tile_pool(name="sb", bufs=3))
    ps = ctx.enter_context(tc.tile_pool(name="ps", bufs=4, space="PSUM"))

    ident = consts.tile([128, 128], BF16)
    make_identity(nc, ident[:])

    wq_s = consts.tile([128, 2, INNER], BF16)
    nc.sync.dma_start(out=wq_s[:], in_=wq.rearrange("kc ck n -> ck kc n"))
    wk_s = consts.tile([128, 2, INNER], BF16)
    nc.sync.dma_start(out=wk_s[:], in_=wk.rearrange("kc ck n -> ck kc n"))
    wv_s = consts.tile([128, 2, INNER], BF16)
    nc.sync.dma_start(out=wv_s[:], in_=wv.rearrange("kc ck n -> ck kc n"))
    wo_s = consts.tile([128, 4, DIM], BF16)
    nc.sync.dma_start(out=wo_s[:], in_=wo.rearrange("kc ck m -> ck kc m"))
    bo_s = consts.tile([128, 2], FP32)
    nc.sync.dma_start(out=bo_s[:], in_=bo.rearrange("(cc p) -> p cc", p=128))

    def ps_slot(shape, dtype=FP32):
        # uniform 2-bank (4KB/partition) psum slots; view-slice to shape
        t = ps.tile([128, 4096 // mybir.dt.size(dtype)], dtype, tag="ps")
        n = int(np.prod(shape[1:]))
        v_ = t[:, 0:n]
        if len(shape) > 2:
            dims = " ".join(f"d{i}" for i in range(1, len(shape)))
            v_ = v_.rearrange(f"p ({dims}) -> p {dims}",
                              **{f"d{i}": shape[i] for i in range(1, len(shape) - 1)})
        return v_

    from collections import deque
    pending = deque()   # out-phase lags two pairs behind

    def drain_one():
        p = pending.popleft()
        pair_out_phase(nc, sb, ps_slot, ident, p)
        if p[2] == 3:
            tile_out_phase(nc, sb, ps_slot, out, wo_s, bo_s, p[1])

    for img in range(n_imgs):
        for wx in range(NW):
            vhat, qT, kT = tile_qkv(nc, sb, ps_slot, fmp, wq_s, wk_s, wv_s,
                                    img, wx)
            outT_tile = sb.tile([128, 4, 512], BF16, tag="outT_tile")
            ts = (outT_tile, img, wx)
            for u in range(4):
                expS = attn_phase_s(nc, sb, ps_slot, qT, kT, u)
                if len(pending) >= 2:
                    drain_one()
                out_tok = attn_phase_av(nc, sb, ps_slot, expS, vhat[u])
                pending.append((out_tok, ts, u))
    while pending:
        drain_one()


def tile_qkv(nc, sb, ps_slot, fmp, wq_s, wk_s, wv_s, img, wx):
    # ---- load fT [c128, ck, t512] ----
    fT = sb.tile([128, 2, 512], BF16, tag="fT")
    nc.sync.dma_start(
        out=fT[:],
        in_=fmp[img, :, :, wx, :, :].rearrange("ck c wy t -> c ck (wy t)"))

    # ---- qT, kT: [n128, t512] x4 chunks ----
    qp = [ps_slot([128, 2, 512]) for _ in range(2)]   # slot holds nk, nk+1
    kp = [ps_slot([128, 2, 512]) for _ in range(2)]
    for half in range(2):
        for sub in range(2):
            nk = 2 * half + sub
            for kc in range(2):
                nc.tensor.matmul(qp[half][:, sub, :],
                                 wq_s[:, kc, 128 * nk:128 * nk + 128],
                                 fT[:, kc, :], start=(kc == 0), stop=(kc == 1))
        for sub in range(2):
            nk = 2 * half + sub
            for kc in range(2):
                nc.tensor.matmul(kp[half][:, sub, :],
                                 wk_s[:, kc, 128 * nk:128 * nk + 128],
                                 fT[:, kc, :], start=(kc == 0), stop=(kc == 1))
    qT = sb.tile([128, 4, 512], BF16, tag="qT")
    kT = sb.tile([128, 4, 512], BF16, tag="kT")
    # q/k evacuation on scalar; vector carries vhat/norm/outT
    nc.scalar.copy(qT[:, 0:2, :], qp[0][:])
    nc.scalar.copy(qT[:, 2:4, :], qp[1][:])
    nc.scalar.copy(kT[:, 0:2, :], kp[0][:])
    nc.scalar.copy(kT[:, 2:4, :], kp[1][:])

    # ---- v: per pair [t128, n512]; vhat adds ones col per head ----
    vhat = []
    for uh in range(2):
        vp = ps_slot([128, 2, 512])   # two pairs per slot? no: [t128, pairsub, n512]
        for us in range(2):
            u = 2 * uh + us
            for kc in range(2):
                nc.tensor.matmul(vp[:, us, :],
                                 fT[:, kc, 128 * u:128 * u + 128],
                                 wv_s[:, kc, :], start=(kc == 0), stop=(kc == 1))
        for us in range(2):
            u = 2 * uh + us
            vh = sb.tile([128, H, D + 1], BF16, tag=f"vhat{u}")
            nc.vector.tensor_copy(
                vh[:, :, 0:D], vp[:, us, :].rearrange("p (h d) -> p h d", h=H))
            nc.gpsimd.memset(vh[:, :, D:D + 1], 1.0)
            vhat.append(vh)

    return vhat, qT, kT


def attn_phase_s(nc, sb, ps_slot, qT, kT, u):
    """Quad-tiled S^T + one full-width exp."""
    # sp[j128, hp, nkx8, i64]: hp stride = 8*64*4B = 2KB -> bank-separated
    sp = ps_slot([128, 2, 8, 64])
    for nk in range(4):
        for hp in range(2):
            for w in range(2):
                o = 128 * u + 64 * w
                nc.tensor.matmul(
                    sp[64 * w:64 * w + 64, hp, nk, :],
                    kT[64 * hp:64 * hp + 64, nk, o:o + 64],
                    qT[64 * hp:64 * hp + 64, nk, o:o + 64],
                    start=True, stop=True, tile_position=(64 * hp, 64 * w))
    expS = sb.tile([128, 2, 4, 64], BF16, tag="expS")
    nc.scalar.activation(expS[:], sp[:, :, 0:4, :], Exp, scale=SCALE)
    return expS


def attn_phase_av(nc, sb, ps_slot, expS, vh):
    """AV + denominator, normalize. Returns out_tok [i128, h, d] bf16."""
    av = ps_slot([128, H, 128])
    for h in range(H):
        nk, hp = h // 2, h % 2
        for w in range(2):
            nc.tensor.matmul(
                av[64 * w:64 * w + 64, h, 0:D + 1],
                expS[64 * w:64 * w + PP, hp, nk, :],
                vh[64 * w:64 * w + PP, h, :],
                start=True, stop=True, tile_position=(64 * w, 64 * w))
    recd = sb.tile([128, H], FP32, tag="recd")
    nc.vector.reciprocal(recd[:], av[:, :, D])
    out_tok = sb.tile([128, H, D], BF16, tag="out_tok")
    nc.vector.tensor_tensor(
        out=out_tok[:], in0=av[:, :, 0:D],
        in1=recd[:].unsqueeze(2).broadcast_to([128, H, D]),
        op=mybir.AluOpType.mult)
    return out_tok


def pair_out_phase(nc, sb, ps_slot, ident, prev):
    """Transpose a previous pair's out_tok into its tile's outT buffer."""
    out_tok, (outT_tile, img, wx), u = prev
    tp = ps_slot([128, 4, 128], BF16)
    ot2 = out_tok[:].rearrange("p h d -> p (h d)")
    for nk in range(4):
        nc.tensor.transpose(tp[:, nk, :], ot2[:, 128 * nk:128 * nk + 128],
                            ident[:])
    nc.vector.tensor_copy(outT_tile[:, :, 128 * u:128 * u + 128], tp[:])


def tile_out_phase(nc, sb, ps_slot, out, wo_s, bo_s, ts):
    """Channel-major out-projection for a whole tile + bias + store."""
    outT_tile, img, wx = ts
    fp_ = ps_slot([128, 2, 512])
    for cc in range(2):
        for nk in range(4):
            nc.tensor.matmul(fp_[:, cc, :],
                             wo_s[:, nk, 128 * cc:128 * cc + 128],
                             outT_tile[:, nk, :],
                             start=(nk == 0), stop=(nk == 3))
    fo = sb.tile([128, 2, 512], BF16, tag="fo")
    for cc in range(2):
        nc.vector.tensor_scalar(
            out=fo[:, cc, :], in0=fp_[:, cc, :],
            scalar1=bo_s[:, cc:cc + 1], scalar2=None,
            op0=mybir.AluOpType.add)
    nc.sync.dma_start(out=out[img, wx], in_=fo[:])


_CACHED = {}


def _get_nc():
    if "nc" not in _CACHED:
        _CACHED["nc"] = build_bass()
    return _CACHED["nc"]


def _marshal_fmap(fmap):
    b = fmap.shape[0]
    A = fmap.reshape(b, NW, P, NW, P, DIM).transpose(0, 5, 1, 3, 2, 4)
    A = np.ascontiguousarray(A).reshape(b, DIM, NW, NW, PP)
    T = np.zeros((b, DIM, NW, NW, 64), dtype=np.float32)
    T[..., :PP] = A
    T = T.reshape(b, 2, 128, NW, NW, 64)
    return T.astype(NPBF16)


def _unmarshal_out(O, b):
    # O: [b, wx, cp128, cc2, t512] bf16, channel-major
    V = O.transpose(0, 1, 3, 2, 4).reshape(b, NW, DIM, 4, 2, 64)[..., :PP]
    V = V.reshape(b, NW, DIM, NW, P, P)          # img, wx, c, wy, p1, p2
    V = V.transpose(0, 1, 4, 3, 5, 2).reshape(b, X, X, DIM)
    return V.astype(np.float32)


def kernel(fmap, Wq, Wkv, Wo, bo, _trace=False, _trace_kwargs=None):
    fmp = _marshal_fmap(np.ascontiguousarray(fmap))
    Wq_ = np.ascontiguousarray(Wq).astype(NPBF16).reshape(2, 128, INNER)
    Wk_ = np.ascontiguousarray(Wkv[:, :INNER]).astype(NPBF16).reshape(2, 128, INNER)
    Wv_ = np.ascontiguousarray(Wkv[:, INNER:]).astype(NPBF16).reshape(2, 128, INNER)
    Wo_ = np.ascontiguousarray(Wo).astype(NPBF16).reshape(4, 128, DIM)
    bo_ = np.ascontiguousarray(bo).astype(np.float32)
    nc = _get_nc()
    in_maps = []
    for c in range(NCORES):
        in_maps.append({
            "fmp": fmp[IMGS_PER_CORE * c:IMGS_PER_CORE * (c + 1)],
            "Wq": Wq_, "Wk": Wk_, "Wv": Wv_, "Wo": Wo_, "bo": bo_,
        })
    res = run_bass_kernel_spmd(nc, in_maps, core_ids=list(range(NCORES)),
                               trace=_trace, **(_trace_kwargs or {}))
    outs = [_unmarshal_out(r["out"], IMGS_PER_CORE) for r in res.results]
    full = np.concatenate(outs, axis=0)
    if _trace:
        return full, res
    return full


# revision 26
# speedup vs baseline: 1.1979x; 1.0708x over previous
"""Local window attention (7x7 windows, 8 heads, d=64) Trainium2 Bass kernel, v3.

Full inputs in, full outputs out. Data-parallel over batch: 4 images/core x 8.
  fmap (32,56,56,256) f32, Wq (256,512), Wkv (256,1024), Wo (512,256), bo (256,)

v3 dataflow (channel-major, host pre-transposed):
  Host packs fmap to fmp[img, ck, c128, wx, wy, t64] bf16 (t = p1*7+p2, 64-slot
  padded windows, zeros in pad) so the kernel DMAs f^T directly - no on-chip
  input transposes.
  Per (img, wx) tile (512 padded tokens = 8 windows = 4 window-pairs):
    qT/kT [n128 x4, t512] = Wq/Wk chunk.T @ fT      (16 MMs, N=512)
    v     [t128, n512] per pair = fT-slice.T @ Wv   (8 MMs, N=512)
    per pair: S^T pair-batched [j128, hp, nk, i128] (8 MMs, N=128,
        row-tiled (64*hp,0); off-diagonal window blocks are garbage, unused)
      exp of the two diagonal blocks (2 ACTs) -> expS [j128, hp, nk, i64] bf16
      AV: av[i64w.., h, 0:65] = expS_w.T @ [v|1]    (16 MMs, N=65,
        quadrant-tiled (64w,64w); col 64 = softmax denominator)
      out_tok = av * recip(denom) broadcast        (1 recip + 1 tensor_tensor)
      outT = PE-transpose(out_tok)                 (4 transposes)
      fin [t128, 256] = outT.T @ Wo + bo           (5 MMs, N=256) -> DMA out
  Host unpacks [img, wx, u, t128, c] bf16 -> (32,56,56,256) f32.
"""

from contextlib import ExitStack

import ml_dtypes
import numpy as np

import concourse.bacc as bacc
import concourse.bass as bass
import concourse.tile as tile
from concourse import mybir
from concourse.masks import make_identity
from concourse.bass_utils import run_bass_kernel_spmd

P = 7
PP = 49
H = 8
D = 64
DIM = 256
INNER = 512
SCALE = D ** -0.5
IMGS_PER_CORE = 4
NCORES = 8
X = 56
NW = X // P      # 8 windows per axis
FP32 = mybir.dt.float32
BF16 = mybir.dt.bfloat16
NPBF16 = ml_dtypes.bfloat16
FP8 = mybir.dt.float8e4
NPFP8 = ml_dtypes.float8_e4m3
WSCALE = 8.0   # fp8 weight prescale; folded back out via exp scale / fo scale
Exp = mybir.ActivationFunctionType.Exp
DR = mybir.MatmulPerfMode.DoubleRow


def build_bass(n_imgs=IMGS_PER_CORE):
    nc = bacc.Bacc("TRN2", target_bir_lowering=False, debug=False)

    fmp = nc.dram_tensor("fmp", [n_imgs, 2, 128, NW, NW, 64], BF16,
                         kind="ExternalInput").ap()
    wq = nc.dram_tensor("Wq", [2, 128, INNER], BF16, kind="ExternalInput").ap()
    wk = nc.dram_tensor("Wk", [2, 128, INNER], BF16, kind="ExternalInput").ap()
    wv = nc.dram_tensor("Wv", [2, 128, INNER], BF16, kind="ExternalInput").ap()
    wo = nc.dram_tensor("Wo", [4, 128, DIM], BF16, kind="ExternalInput").ap()
    bo = nc.dram_tensor("bo", [DIM], FP32, kind="ExternalInput").ap()
    out = nc.dram_tensor("out", [n_imgs, NW, 128, 2, 512], BF16,
                         kind="ExternalOutput").ap()

    with tile.TileContext(nc) as tc:
        with ExitStack() as ctx:
            build_kernel(ctx, tc, out, fmp, wq, wk, wv, wo, bo, n_imgs)
    nc.compile()
    return nc


def build_kernel(ctx, tc, out, fmp, wq, wk, wv, wo, bo, n_imgs):
    nc = tc.nc
    consts = ctx.enter_context(tc.tile_pool(name="consts", bufs=1))
    sb = ctx.enter_context(tc.tile_pool(name="sb", bufs=3))
    ps = ctx.enter_context(tc.tile_pool(name="ps", bufs=4, space="PSUM"))

    ident = consts.tile([128, 128], BF16)
    make_identity(nc, ident[:])

    wq_s = consts.tile([128, 2, INNER], BF16)
    nc.sync.dma_start(out=wq_s[:], in_=wq.rearrange("kc ck n -> ck kc n"))
    wk_s = consts.tile([128, 2, INNER], BF16)
    nc.sync.dma_start(out=wk_s[:], in_=wk.rearrange("kc ck n -> ck kc n"))
    wv_s = consts.tile([128, 2, INNER], BF16)
    nc.sync.dma_start(out=wv_s[:], in_=wv.rearrange("kc ck n -> ck kc n"))
    wo_s = consts.tile([128, 4, DIM], BF16)
    nc.sync.dma_start(out=wo_s[:], in_=wo.rearrange("kc ck m -> ck kc m"))
    bo_s = consts.tile([128, 2], FP32)
    nc.sync.dma_start(out=bo_s[:], in_=bo.rearrange("(cc p) -> p cc", p=128))

    def ps_slot(shape, dtype=FP32):
        # uniform 2-bank (4KB/partition) psum slots; view-slice to shape
        t = ps.tile([128, 4096 // mybir.dt.size(dtype)], dtype, tag="ps")
        n = int(np.prod(shape[1:]))
        v_ = t[:, 0:n]
        if len(shape) > 2:
            dims = " ".join(f"d{i}" for i in range(1, len(shape)))
            v_ = v_.rearrange(f"p ({dims}) -> p {dims}",
                              **{f"d{i}": shape[i] for i in range(1, len(shape) - 1)})
        return v_

    from collections import deque
    pending = deque()   # out-phase lags two pairs behind

    def drain_one():
        p = pending.popleft()
        pair_out_phase(nc, sb, ps_slot, ident, p)
        if p[2] == 3:
            tile_out_phase(nc, sb, ps_slot, out, wo_s, bo_s, p[1])

    for img in range(n_imgs):
        for wx in range(NW):
            vhat, qT, kT = tile_qkv(nc, sb, ps_slot, fmp, wq_s, wk_s, wv_s,
                                    img, wx)
            outT_tile = sb.tile([128, 4, 512], BF16, tag="outT_tile")
            ts = (outT_tile, img, wx)
            for u in range(4):
                expS = attn_phase_s(nc, sb, ps_slot, qT, kT, u)
                if len(pending) >= 2:
                    drain_one()
                out_tok = attn_phase_av(nc, sb, ps_slot, expS, vhat[u])
                pending.append((out_tok, ts, u))
    while pending:
        drain_one()


def tile_qkv(nc, sb, ps_slot, fmp, wq_s, wk_s, wv_s, img, wx):
    # ---- load fT [c128, ck, t512] ----
    fT = sb.tile([128, 2, 512], BF16, tag="fT")
    nc.sync.dma_start(
        out=fT[:],
        in_=fmp[img, :, :, wx, :, :].rearrange("ck c wy t -> c ck (wy t)"))

    # ---- qT, kT: [n128, t512] x4 chunks ----
    qp = [ps_slot([128, 2, 512]) for _ in range(2)]   # slot holds nk, nk+1
    kp = [ps_slot([128, 2, 512]) for _ in range(2)]
    for half in range(2):
        for sub in range(2):
            nk = 2 * half + sub
            for kc in range(2):
                nc.tensor.matmul(qp[half][:, sub, :],
                                 wq_s[:, kc, 128 * nk:128 * nk + 128],
                                 fT[:, kc, :], start=(kc == 0), stop=(kc == 1))
        for sub in range(2):
            nk = 2 * half + sub
            for kc in range(2):
                nc.tensor.matmul(kp[half][:, sub, :],
                                 wk_s[:, kc, 128 * nk:128 * nk + 128],
                                 fT[:, kc, :], start=(kc == 0), stop=(kc == 1))
    qT = sb.tile([128, 4, 512], BF16, tag="qT")
    kT = sb.tile([128, 4, 512], BF16, tag="kT")
    # q/k evacuation on scalar; vector carries vhat/norm/outT
    nc.scalar.copy(qT[:, 0:2, :], qp[0][:])
    nc.scalar.copy(qT[:, 2:4, :], qp[1][:])
    nc.scalar.copy(kT[:, 0:2, :], kp[0][:])
    nc.scalar.copy(kT[:, 2:4, :], kp[1][:])

    # ---- v: per pair [t128, n512]; vhat adds ones col per head ----
    vhat = []
    for uh in range(2):
        vp = ps_slot([128, 2, 512])   # two pairs per slot? no: [t128, pairsub, n512]
        for us in range(2):
            u = 2 * uh + us
            for kc in range(2):
                nc.tensor.matmul(vp[:, us, :],
                                 fT[:, kc, 128 * u:128 * u + 128],
                                 wv_s[:, kc, :], start=(kc == 0), stop=(kc == 1))
        for us in range(2):
            u = 2 * uh + us
            vh = sb.tile([128, H, D + 1], BF16, tag=f"vhat{u}")
            nc.vector.tensor_copy(
                vh[:, :, 0:D], vp[:, us, :].rearrange("p (h d) -> p h d", h=H))
            nc.gpsimd.memset(vh[:, :, D:D + 1], 1.0)
            vhat.append(vh)

    return vhat, qT, kT


def attn_phase_s(nc, sb, ps_slot, qT, kT, u):
    """Quad-tiled S^T + one full-width exp."""
    # sp[j128, hp, nkx8, i64]: hp stride = 8*64*4B = 2KB -> bank-separated
    sp = ps_slot([128, 2, 8, 64])
    for nk in range(4):
        for hp in range(2):
            for w in range(2):
                o = 128 * u + 64 * w
                nc.tensor.matmul(
                    sp[64 * w:64 * w + 64, hp, nk, 0:PP],
                    kT[64 * hp:64 * hp + 64, nk, o:o + 64],
                    qT[64 * hp:64 * hp + 64, nk, o:o + PP],
                    start=True, stop=True, tile_position=(64 * hp, 64 * w))
    expS = sb.tile([128, 2, 4, 64], BF16, tag="expS")
    nc.scalar.activation(expS[:, :, :, 0:PP], sp[:, :, 0:4, 0:PP], Exp,
                         scale=SCALE)
    return expS


def attn_phase_av(nc, sb, ps_slot, expS, vh):
    """AV + denominator, normalize. Returns out_tok [i128, h, d] bf16."""
    av = ps_slot([128, H, 128])
    for h in range(H):
        nk, hp = h // 2, h % 2
        for w in range(2):
            nc.tensor.matmul(
                av[64 * w:64 * w + 64, h, 0:D + 1],
                expS[64 * w:64 * w + PP, hp, nk, :],
                vh[64 * w:64 * w + PP, h, :],
                start=True, stop=True, tile_position=(64 * w, 64 * w))
    recd = sb.tile([128, H], FP32, tag="recd")
    nc.vector.reciprocal(recd[:], av[:, :, D])
    out_tok = sb.tile([128, H, D], BF16, tag="out_tok")
    nc.vector.tensor_tensor(
        out=out_tok[:], in0=av[:, :, 0:D],
        in1=recd[:].unsqueeze(2).broadcast_to([128, H, D]),
        op=mybir.AluOpType.mult)
    return out_tok


def pair_out_phase(nc, sb, ps_slot, ident, prev):
    """Transpose a previous pair's out_tok into its tile's outT buffer."""
    out_tok, (outT_tile, img, wx), u = prev
    tp = ps_slot([128, 4, 128], BF16)
    ot2 = out_tok[:].rearrange("p h d -> p (h d)")
    for nk in range(4):
        nc.tensor.transpose(tp[:, nk, :], ot2[:, 128 * nk:128 * nk + 128],
                            ident[:])
    nc.vector.tensor_copy(outT_tile[:, :, 128 * u:128 * u + 128], tp[:])


def tile_out_phase(nc, sb, ps_slot, out, wo_s, bo_s, ts):
    """Channel-major out-projection for a whole tile + bias + store."""
    outT_tile, img, wx = ts
    fp_ = ps_slot([128, 2, 512])
    for cc in range(2):
        for nk in range(4):
            nc.tensor.matmul(fp_[:, cc, :],
                             wo_s[:, nk, 128 * cc:128 * cc + 128],
                             outT_tile[:, nk, :],
                             start=(nk == 0), stop=(nk == 3))
    fo = sb.tile([128, 2, 512], BF16, tag="fo")
    for cc in range(2):
        nc.vector.tensor_scalar(
            out=fo[:, cc, :], in0=fp_[:, cc, :],
            scalar1=bo_s[:, cc:cc + 1], scalar2=None,
            op0=mybir.AluOpType.add)
    nc.sync.dma_start(out=out[img, wx], in_=fo[:])


_CACHED = {}


def _get_nc():
    if "nc" not in _CACHED:
        _CACHED["nc"] = build_bass()
    return _CACHED["nc"]


def _marshal_fmap(fmap):
    b = fmap.shape[0]
    A = fmap.reshape(b, NW, P, NW, P, DIM).transpose(0, 5, 1, 3, 2, 4)
    A = np.ascontiguousarray(A).reshape(b, DIM, NW, NW, PP)
    T = np.zeros((b, DIM, NW, NW, 64), dtype=np.float32)
    T[..., :PP] = A
    T = T.reshape(b, 2, 128, NW, NW, 64)
    return T.astype(NPBF16)


def _unmarshal_out(O, b):
    # O: [b, wx, cp128, cc2, t512] bf16, channel-major
    V = O.transpose(0, 1, 3, 2, 4).reshape(b, NW, DIM, 4, 2, 64)[..., :PP]
    V = V.reshape(b, NW, DIM, NW, P, P)          # img, wx, c, wy, p1, p2
    V = V.transpose(0, 1, 4, 3, 5, 2).reshape(b, X, X, DIM)
    return V.astype(np.float32)


def kernel(fmap, Wq, Wkv, Wo, bo, _trace=False, _trace_kwargs=None):
    fmp = _marshal_fmap(np.ascontiguousarray(fmap))
    Wq_ = np.ascontiguousarray(Wq).astype(NPBF16).reshape(2, 128, INNER)
    Wk_ = np.ascontiguousarray(Wkv[:, :INNER]).astype(NPBF16).reshape(2, 128, INNER)
    Wv_ = np.ascontiguousarray(Wkv[:, INNER:]).astype(NPBF16).reshape(2, 128, INNER)
    Wo_ = np.ascontiguousarray(Wo).astype(NPBF16).reshape(4, 128, DIM)
    bo_ = np.ascontiguousarray(bo).astype(np.float32)
    nc = _get_nc()
    in_maps = []
    for c in range(NCORES):
        in_maps.append({
            "fmp": fmp[IMGS_PER_CORE * c:IMGS_PER_CORE * (c + 1)],
            "Wq": Wq_, "Wk": Wk_, "Wv": Wv_, "Wo": Wo_, "bo": bo_,
        })
    res = run_bass_kernel_spmd(nc, in_maps, core_ids=list(range(NCORES)),
                               trace=_trace, **(_trace_kwargs or {}))
    outs = [_unmarshal_out(r["out"], IMGS_PER_CORE) for r in res.results]
    full = np.concatenate(outs, axis=0)
    if _trace:
        return full, res
    return full
